# revision 1
# baseline (speedup 1.0000x reference)
"""Causal self-attention (single-head, d=1024, seq=4096, batch=4) on 8 TRN2 cores.

Sharding: core c = (batch b = c//2, key-parity h = c%2). Each core computes
partial (unnormalized) attention for ALL queries of its batch element over
half the keys — the alternating 128-key blocks j = 2t+h, host-permuted into a
contiguous local key tensor. Partials combine exactly on the host:
out = (num0 + num1) / (den0 + den1). No softmax max-subtraction: logits are
|q.k|/32 <~ 3 for this input distribution, so exp never overflows and the
partial-sum combine is exact.

All matmuls run in float32r (full PE rate at moving-dim >= 256, ~1.4e-4
relative error). Host pre-arranges every input into the exact SBUF tile
layout (chunk-major x, quarter-major W) so each DMA is long contiguous runs.

Device program (identical SPMD program on all 8 cores; per-core variation is
input data only):
  - K/V projections of the 2048 local keys in half-passes (K by output
    column half, V by d_out half), streaming x^T chunks boustrophedon through
    4 LRU slots so pass reversals reuse hot chunks; each weight half-slot
    frees one half-pass early so the next load overlaps compute.
  - Per 256-query block g: project Q^T on the fly, then for t = 0..g:
    scores S^T[k128, q256] = KT.T @ QT (8 accumulating matmuls), exp via ACT
    (scale=1/32) straight out of PSUM into f32r SBUF, causal mask multiply on
    the last trip, denominator via an M=1 ones-stationary matmul, and AV
    accumulation into 4 PSUM banks [q128, o512].
"""

import numpy as np

import concourse.bacc as bacc
import concourse.tile as tile
import concourse.mybir as mybir
from concourse.bass_utils import run_bass_kernel_spmd

D = 1024
DB = D // 128  # 8 d-blocks (contraction tiles)
QW = 256  # query-block width (scores moving free dim; >=256 keeps f32r full-rate)
F32 = mybir.dt.float32
F32R = mybir.dt.float32r


def build_program(seq, num_devices):
    NG = seq // QW  # query blocks per core (all queries)
    NKL = seq // 2  # local keys per core
    NKB = NKL // 128  # local key blocks; == NG
    KC = min(256, NKL)  # xk stream chunk width (columns of x^T)
    NCH = NKL // KC

    nc = bacc.Bacc("TRN2", target_bir_lowering=False, debug=False,
                   num_devices=num_devices)

    # Inputs are host-side rearranged into device tile layout:
    #   xq [NG, 128, DB, QW], xk [NCH, 128, DB, KC]  (x^T chunk-major)
    #   wq/wk/wv [8, 128, DB, 128]                   (W^T quarter-major)
    xq = nc.dram_tensor("xq", [NG, 128, DB, QW], F32R, kind="ExternalInput")
    xk = nc.dram_tensor("xk", [NCH, 128, DB, KC], F32R, kind="ExternalInput")
    wq = nc.dram_tensor("wq", [8, 128, DB, 128], F32R, kind="ExternalInput")
    wk = nc.dram_tensor("wk", [8, 128, DB, 128], F32R, kind="ExternalInput")
    wv = nc.dram_tensor("wv", [8, 128, DB, 128], F32R, kind="ExternalInput")
    mask = nc.dram_tensor("mask", [128, QW], F32R, kind="ExternalInput")
    num = nc.dram_tensor("num", [seq, D], F32, kind="ExternalOutput")
    den = nc.dram_tensor("den", [1, seq], F32, kind="ExternalOutput")

    with tile.TileContext(nc) as tc:
        with (
            tc.tile_pool(name="res", bufs=1) as res,
            tc.tile_pool(name="wpool", bufs=1) as wpool,
            tc.tile_pool(name="qts", bufs=1) as qts,
            tc.tile_pool(name="pp", bufs=2) as pp,
            tc.tile_pool(name="outp", bufs=2) as outp,
            tc.tile_pool(name="pss", bufs=2, space="PSUM") as pss,
            tc.tile_pool(name="psav", bufs=5, space="PSUM") as psav,
            tc.tile_pool(name="psden", bufs=1, space="PSUM") as psden,
        ):
            kt = res.tile([128, DB, NKL], F32R, tag="kt")
            vv = res.tile([128, NKB, D], F32R, tag="vv")
            mk = res.tile([128, QW], F32R, tag="mk")
            ones_f = res.tile([128, 1], F32, tag="onesf")
            ones_r = res.tile([128, 1], F32R, tag="onesr")

            # ---- chunk slots: explicit LRU rotation ----
            nslots = min(4, max(2, NCH))
            chslots = [res.tile([128, DB, KC], F32R, tag=f"ch{i}", name=f"ch{i}")
                       for i in range(nslots)]
            chstate = {"live": {}, "clock": 0, "lastuse": {}}

            def get_chunk(key, src_ap):
                live, lastuse = chstate["live"], chstate["lastuse"]
                chstate["clock"] += 1
                if key in live:
                    lastuse[live[key]] = chstate["clock"]
                    return chslots[live[key]]
                # evict the least-recently-USED slot: its readers finish
                # earliest, so the refill DMA starts earliest
                slot = min(range(nslots), key=lambda i: lastuse.get(i, -1))
                for k2 in [k2 for k2, s2 in live.items() if s2 == slot]:
                    del live[k2]
                live[key] = slot
                lastuse[slot] = chstate["clock"]
                nc.sync.dma_start(chslots[slot][:], src_ap)
                return chslots[slot]

            def w_half(wsrc, oh, nm, eng, qrange=range(4)):
                wt = wpool.tile([128, DB, 512], F32R, tag=f"w{nm[-1]}", name=nm)
                for q in qrange:
                    eng.dma_start(wt[:, :, q * 128:(q + 1) * 128],
                                  wsrc.ap()[oh * 4 + q])
                return wt

            # ---- projections in half-passes with boustrophedon chunks ----
            def k_pass(wt, oh, order, pi):
                for kc in order:
                    xt = get_chunk(kc, xk.ap()[kc])
                    for obh in range(4):
                        ob = oh * 4 + obh
                        acc = pss.tile([128, KC], F32, tag="s",
                                       name=f"acck_{pi}_{kc}_{obh}")
                        for db in range(DB):
                            nc.tensor.matmul(
                                acc[:], wt[:, db, obh * 128:(obh + 1) * 128],
                                xt[:, db, :], start=(db == 0), stop=(db == DB - 1))
                        nc.vector.tensor_copy(kt[:, ob, kc * KC:(kc + 1) * KC], acc[:])

            def v_pass(wt, oh, order, pi):
                for kc in order:
                    xt = get_chunk(kc, xk.ap()[kc])
                    for nb in range(KC // 128):
                        kb = kc * (KC // 128) + nb
                        acc = pss.tile([128, 512], F32, tag="s",
                                       name=f"accv_{pi}_{kc}_{nb}")
                        for db in range(DB):
                            nc.tensor.matmul(
                                acc[:], xt[:, db, nb * 128:(nb + 1) * 128],
                                wt[:, db, :], start=(db == 0), stop=(db == DB - 1))
                        nc.vector.tensor_copy(
                            vv[:, kb, oh * 512:(oh + 1) * 512], acc[:])

            fwd = list(range(NCH))
            rev = fwd[::-1]
            # startup: interleave the first weight quarter with chunk 0 on
            # the sync queue so the first matmul chain starts ~4us in
            wk_lo = w_half(wk, 0, "wk_A", nc.sync, qrange=[0])
            get_chunk(0, xk.ap()[0])
            for q in range(1, 4):
                nc.sync.dma_start(wk_lo[:, :, q * 128:(q + 1) * 128],
                                  wk.ap()[q])
                if q < NCH and nslots > q:
                    get_chunk(q, xk.ap()[q])
            wk_hi = w_half(wk, 1, "wk_B", nc.gpsimd)
            k_pass(wk_lo, 0, fwd, 0)
            wv_lo = w_half(wv, 0, "wv_A", nc.scalar)  # A freed by klo end
            k_pass(wk_hi, 1, rev, 1)
            wv_hi = w_half(wv, 1, "wv_B", nc.scalar)
            v_pass(wv_lo, 0, fwd, 2)
            wqa = w_half(wq, 0, "wq_A", nc.scalar)
            v_pass(wv_hi, 1, rev, 3)
            wqb = w_half(wq, 1, "wq_B", nc.sync)

            nc.sync.dma_start(mk[:], mask.ap())
            nc.vector.memset(ones_f[:], 1.0)
            nc.vector.tensor_copy(ones_r[:], ones_f[:])

            # ---- attention over query blocks ----
            for g in range(NG):
                xt = get_chunk(("q", g), xq.ap()[g])
                qt = qts.tile([128, DB, QW], F32R, tag="qt")
                for ob in range(DB):
                    wt = wqa if ob < 4 else wqb
                    obh = ob % 4
                    accq = pss.tile([128, QW], F32, tag="s", name=f"accq_{g}_{ob}")
                    for db in range(DB):
                        nc.tensor.matmul(
                            accq[:], wt[:, db, obh * 128:(obh + 1) * 128],
                            xt[:, db, :], start=(db == 0), stop=(db == DB - 1))
                    if ob % 2 == 0:
                        nc.scalar.copy(qt[:, ob, :], accq[:])
                    else:
                        nc.vector.tensor_copy(qt[:, ob, :], accq[:])

                av = [psav.tile([128, 512], F32, tag="av", name=f"av_{g}_{i}")
                      for i in range(4)]
                dn = psden.tile([1, QW], F32, tag="den", name=f"dn_{g}")

                for t in range(g + 1):
                    accs = pss.tile([128, QW], F32, tag="s")
                    for ob in range(DB):
                        nc.tensor.matmul(
                            accs[:], kt[:, ob, t * 128:(t + 1) * 128],
                            qt[:, ob, :], start=(ob == 0), stop=(ob == DB - 1))
                    pt = pp.tile([128, QW], F32R, tag="p")
                    nc.scalar.activation(
                        pt[:], accs[:], mybir.ActivationFunctionType.Exp,
                        scale=0.03125)
                    if t == g:
                        nc.vector.tensor_mul(pt[:], pt[:], mk[:])
                    nc.tensor.matmul(
                        dn[:], ones_r[:], pt[:],
                        start=(t == 0), stop=(t == g))
                    for qs in range(2):
                        psub = pt[:, qs * 128:(qs + 1) * 128]
                        for dh in range(2):
                            nc.tensor.matmul(
                                av[qs * 2 + dh][:], psub,
                                vv[:, t, dh * 512:(dh + 1) * 512],
                                start=(t == 0), stop=(t == g))

                for qs in range(2):
                    row = g * QW + qs * 128
                    for dh in range(2):
                        st = outp.tile([128, 512], F32, tag="numst",
                                       name=f"st_{g}_{qs}_{dh}")
                        if dh == 0:
                            nc.vector.tensor_copy(st[:], av[qs * 2 + dh][:])
                        else:
                            nc.scalar.copy(st[:], av[qs * 2 + dh][:])
                        eng = nc.sync if dh == 0 else nc.scalar
                        eng.dma_start(
                            num.ap()[row:row + 128, dh * 512:(dh + 1) * 512], st[:])
                dtmp = outp.tile([1, QW], F32, tag="numst", name=f"dtmp_{g}")
                nc.vector.tensor_copy(dtmp[:], dn[:])
                nc.gpsimd.dma_start(den.ap()[:, g * QW:(g + 1) * QW], dtmp[:])

    nc.compile()
    return nc


def _chunks(a, w):
    """[1024, n] (d-major) -> [n//w, 128, DB, w] chunk-major tile layout:
    element (c, p, db, j) = a[db*128 + p, c*w + j]."""
    d, n = a.shape
    return np.ascontiguousarray(
        a.reshape(DB, 128, n // w, w).transpose(2, 1, 0, 3))


def make_core_inputs(x, wqT, wkT, wvT, seq):
    """Per-core in_maps for batch elements of x [B, seq, d]."""
    NKB = seq // 256
    wq_d = _chunks(wqT, 128)
    wk_d = _chunks(wkT, 128)
    wv_d = _chunks(wvT, 128)
    masks = []
    for h in range(2):
        kk = np.arange(128)[:, None]
        qq = np.arange(QW)[None, :]
        masks.append((kk + 128 * h <= qq).astype(np.float32))
    in_maps = []
    for b in range(x.shape[0]):
        xT = np.ascontiguousarray(x[b].T)  # [d, seq]
        xq_d = _chunks(xT, QW)
        for h in range(2):
            cols = np.concatenate(
                [np.arange((2 * t + h) * 128, (2 * t + h + 1) * 128)
                 for t in range(NKB)])
            xk_d = _chunks(np.ascontiguousarray(xT[:, cols]),
                           min(256, seq // 2))
            in_maps.append({
                "xq": xq_d, "xk": xk_d, "wq": wq_d, "wk": wk_d, "wv": wv_d,
                "mask": masks[h],
            })
    return in_maps


_prog_cache = {}


def _get_program(seq, num_devices):
    key = (seq, num_devices)
    if key not in _prog_cache:
        _prog_cache[key] = build_program(seq, num_devices)
    return _prog_cache[key]


def combine_partials(results, batch, seq):
    out = np.empty((batch, seq, D), dtype=np.float32)
    for b in range(batch):
        r0, r1 = results[2 * b], results[2 * b + 1]
        num = r0["num"].astype(np.float64) + r1["num"].astype(np.float64)
        den_flat = (r0["den"].astype(np.float64)
                    + r1["den"].astype(np.float64)).reshape(-1)
        out[b] = (num / den_flat[:, None]).astype(np.float32)
    return out


def kernel(x, Wq, Wk, Wv):
    x = np.asarray(x, dtype=np.float32)
    batch, seq, d = x.shape
    assert d == D
    wqT = np.ascontiguousarray(np.asarray(Wq, dtype=np.float32).T)
    wkT = np.ascontiguousarray(np.asarray(Wk, dtype=np.float32).T)
    wvT = np.ascontiguousarray(np.asarray(Wv, dtype=np.float32).T)
    n_cores = 2 * batch
    nc = _get_program(seq, n_cores)
    in_maps = make_core_inputs(x, wqT, wkT, wvT, seq)
    res = run_bass_kernel_spmd(nc, in_maps, core_ids=list(range(n_cores)))
    return combine_partials(res.results, batch, seq)



# revision 2
# speedup vs baseline: 1.3435x; 1.3435x over previous
"""Causal self-attention (single-head, d=1024, seq=4096, batch=4) on 8 TRN2 cores.

Sharding: core c = (batch b = c//2, key-parity h = c%2). Each core computes
partial (unnormalized) attention for ALL queries of its batch element over
half the keys — the alternating 128-key blocks j = 2t+h, host-permuted into a
contiguous local key tensor. Partials combine exactly on the host:
out = (num0 + num1) / (den0 + den1). No softmax max-subtraction: logits are
|q.k|/32 <~ 3 for this input distribution, so exp never overflows and the
partial-sum combine is exact.

Dtype strategy (measured on this part: bf16 matmul streams at full 2.35 GHz
with hidden FWL weight loads, while f32r pays a separate ~equal-length
LDWEIGHTS; fp8e4 DoubleRow doubles the FLOP rate):
  - x and all weights in bf16 (host-converted); projections accumulate f32.
  - Q^T and K^T are written from PSUM as fp8e4; the scores matmul runs as
    4 DoubleRow matmuls (256-deep contraction each) at 2x rate.
  - V, P (exp scores) in bf16; AV + denominator accumulate in f32 PSUM.
End-to-end rel err ~1.3e-2 (CPU-validated), inside the 2e-2 gate.

Device program (identical SPMD program on all 8 cores; per-core variation is
input data only):
  - K/V projections of the 2048 local keys in half-passes (K by output
    column half, V by d_out half), streaming x^T chunks boustrophedon through
    4 LRU slots so pass reversals reuse hot chunks; each weight half-slot
    frees one half-pass early so the next load overlaps compute.
  - Per 256-query block g: project Q^T on the fly, then for t = 0..g:
    scores S^T[k128, q256] = KT.T @ QT (4 fp8 DoubleRow matmuls), exp via ACT
    (scale=1/32) straight out of PSUM into bf16 SBUF, causal mask multiply on
    the last trip, denominator via an M=1 ones-stationary matmul, and AV
    accumulation into 4 PSUM banks [q128, o512].
"""

import numpy as np
import ml_dtypes

import concourse.bacc as bacc
import concourse.tile as tile
import concourse.mybir as mybir
from concourse.bass_utils import run_bass_kernel_spmd

D = 1024
DB = D // 128  # 8 d-blocks (contraction tiles)
QW = 256  # query-block width (scores moving free dim)
F32 = mybir.dt.float32
BF16 = mybir.dt.bfloat16
FP8 = mybir.dt.float8e4
DR = mybir.MatmulPerfMode.DoubleRow
BF16_NP = ml_dtypes.bfloat16


def build_program(seq, num_devices):
    NG = seq // QW  # query blocks per core (all queries)
    NKL = seq // 2  # local keys per core
    NKB = NKL // 128  # local key blocks; == NG
    KC = min(256, NKL)  # xk stream chunk width (columns of x^T)
    NCH = NKL // KC

    nc = bacc.Bacc("TRN2", target_bir_lowering=False, debug=False,
                   num_devices=num_devices)

    # Inputs are host-side rearranged into device tile layout:
    #   xq [NG, 128, DB, QW], xk [NCH, 128, DB, KC]  (x^T chunk-major)
    #   wq/wk/wv [8, 128, DB, 128]                   (W^T quarter-major)
    xq = nc.dram_tensor("xq", [NG, 128, DB, QW], BF16, kind="ExternalInput")
    xk = nc.dram_tensor("xk", [NCH, 128, DB, KC], BF16, kind="ExternalInput")
    wq = nc.dram_tensor("wq", [8, 128, DB, 128], BF16, kind="ExternalInput")
    wk = nc.dram_tensor("wk", [8, 128, DB, 128], BF16, kind="ExternalInput")
    wv = nc.dram_tensor("wv", [8, 128, DB, 128], BF16, kind="ExternalInput")
    mask = nc.dram_tensor("mask", [128, QW], BF16, kind="ExternalInput")
    num = nc.dram_tensor("num", [seq, D], F32, kind="ExternalOutput")
    den = nc.dram_tensor("den", [1, seq], F32, kind="ExternalOutput")

    with tile.TileContext(nc) as tc:
        with (
            tc.tile_pool(name="res", bufs=1) as res,
            tc.tile_pool(name="wpool", bufs=1) as wpool,
            tc.tile_pool(name="qts", bufs=1) as qts,
            tc.tile_pool(name="pp", bufs=2) as pp,
            tc.tile_pool(name="outp", bufs=2) as outp,
            tc.tile_pool(name="pss", bufs=2, space="PSUM") as pss,
            tc.tile_pool(name="psav", bufs=5, space="PSUM") as psav,
            tc.tile_pool(name="psden", bufs=1, space="PSUM") as psden,
        ):
            kt = res.tile([128, DB, NKL], FP8, tag="kt")
            vv = res.tile([128, NKB, D], BF16, tag="vv")
            mk = res.tile([128, QW], BF16, tag="mk")
            ones_f = res.tile([128, 1], F32, tag="onesf")
            ones_b = res.tile([128, 1], BF16, tag="onesr")

            # ---- chunk slots: explicit LRU rotation ----
            nslots = min(4, max(2, NCH))
            chslots = [res.tile([128, DB, KC], BF16, tag=f"ch{i}", name=f"ch{i}")
                       for i in range(nslots)]
            chstate = {"live": {}, "clock": 0, "lastuse": {}}

            def get_chunk(key, src_ap):
                live, lastuse = chstate["live"], chstate["lastuse"]
                chstate["clock"] += 1
                if key in live:
                    lastuse[live[key]] = chstate["clock"]
                    return chslots[live[key]]
                # evict the least-recently-USED slot: its readers finish
                # earliest, so the refill DMA starts earliest
                slot = min(range(nslots), key=lambda i: lastuse.get(i, -1))
                for k2 in [k2 for k2, s2 in live.items() if s2 == slot]:
                    del live[k2]
                live[key] = slot
                lastuse[slot] = chstate["clock"]
                nc.sync.dma_start(chslots[slot][:], src_ap)
                return chslots[slot]

            def w_half(wsrc, oh, nm, eng, qrange=range(4)):
                wt = wpool.tile([128, DB, 512], BF16, tag=f"w{nm[-1]}", name=nm)
                for q in qrange:
                    eng.dma_start(wt[:, :, q * 128:(q + 1) * 128],
                                  wsrc.ap()[oh * 4 + q])
                return wt

            # ---- projections in half-passes with boustrophedon chunks ----
            def k_pass(wt, oh, order, pi):
                for kc in order:
                    xt = get_chunk(kc, xk.ap()[kc])
                    for obh in range(4):
                        ob = oh * 4 + obh
                        acc = pss.tile([128, KC], F32, tag="s",
                                       name=f"acck_{pi}_{kc}_{obh}")
                        for db in range(DB):
                            nc.tensor.matmul(
                                acc[:], wt[:, db, obh * 128:(obh + 1) * 128],
                                xt[:, db, :], start=(db == 0), stop=(db == DB - 1))
                        nc.vector.tensor_copy(kt[:, ob, kc * KC:(kc + 1) * KC], acc[:])

            def v_pass(wt, oh, order, pi):
                for kc in order:
                    xt = get_chunk(kc, xk.ap()[kc])
                    for nb in range(KC // 128):
                        kb = kc * (KC // 128) + nb
                        acc = pss.tile([128, 512], F32, tag="s",
                                       name=f"accv_{pi}_{kc}_{nb}")
                        for db in range(DB):
                            nc.tensor.matmul(
                                acc[:], xt[:, db, nb * 128:(nb + 1) * 128],
                                wt[:, db, :], start=(db == 0), stop=(db == DB - 1))
                        nc.vector.tensor_copy(
                            vv[:, kb, oh * 512:(oh + 1) * 512], acc[:])

            fwd = list(range(NCH))
            rev = fwd[::-1]
            # startup: interleave the first weight quarter with chunk 0 on
            # the sync queue so the first matmul chain starts ~4us in
            wk_lo = w_half(wk, 0, "wk_A", nc.sync, qrange=[0])
            get_chunk(0, xk.ap()[0])
            for q in range(1, 4):
                nc.sync.dma_start(wk_lo[:, :, q * 128:(q + 1) * 128],
                                  wk.ap()[q])
                if q < NCH and nslots > q:
                    get_chunk(q, xk.ap()[q])
            wk_hi = w_half(wk, 1, "wk_B", nc.gpsimd)
            k_pass(wk_lo, 0, fwd, 0)
            wv_lo = w_half(wv, 0, "wv_A", nc.scalar)  # A freed by klo end
            k_pass(wk_hi, 1, rev, 1)
            wv_hi = w_half(wv, 1, "wv_B", nc.scalar)
            v_pass(wv_lo, 0, fwd, 2)
            wqa = w_half(wq, 0, "wq_A", nc.scalar)
            v_pass(wv_hi, 1, rev, 3)
            wqb = w_half(wq, 1, "wq_B", nc.sync)

            nc.sync.dma_start(mk[:], mask.ap())
            nc.vector.memset(ones_f[:], 1.0)
            nc.vector.tensor_copy(ones_b[:], ones_f[:])

            # ---- attention over query blocks ----
            for g in range(NG):
                xt = get_chunk(("q", g), xq.ap()[g])
                qt = qts.tile([128, DB, QW], FP8, tag="qt")
                for ob in range(DB):
                    wt = wqa if ob < 4 else wqb
                    obh = ob % 4
                    accq = pss.tile([128, QW], F32, tag="s", name=f"accq_{g}_{ob}")
                    for db in range(DB):
                        nc.tensor.matmul(
                            accq[:], wt[:, db, obh * 128:(obh + 1) * 128],
                            xt[:, db, :], start=(db == 0), stop=(db == DB - 1))
                    if ob % 2 == 0:
                        nc.scalar.copy(qt[:, ob, :], accq[:])
                    else:
                        nc.vector.tensor_copy(qt[:, ob, :], accq[:])

                av = [psav.tile([128, 512], F32, tag="av", name=f"av_{g}_{i}")
                      for i in range(4)]
                dn = psden.tile([1, QW], F32, tag="den", name=f"dn_{g}")

                for t in range(g + 1):
                    accs = pss.tile([128, QW], F32, tag="s")
                    for i in range(4):
                        nc.tensor.matmul(
                            accs[:], kt[:, 2 * i:2 * i + 2, t * 128:(t + 1) * 128],
                            qt[:, 2 * i:2 * i + 2, :],
                            start=(i == 0), stop=(i == 3), perf_mode=DR)
                    pt = pp.tile([128, QW], BF16, tag="p")
                    nc.scalar.activation(
                        pt[:], accs[:], mybir.ActivationFunctionType.Exp,
                        scale=0.03125)
                    if t == g:
                        nc.vector.tensor_mul(pt[:], pt[:], mk[:])
                    nc.tensor.matmul(
                        dn[:], ones_b[:], pt[:],
                        start=(t == 0), stop=(t == g))
                    for qs in range(2):
                        psub = pt[:, qs * 128:(qs + 1) * 128]
                        for dh in range(2):
                            nc.tensor.matmul(
                                av[qs * 2 + dh][:], psub,
                                vv[:, t, dh * 512:(dh + 1) * 512],
                                start=(t == 0), stop=(t == g))

                for qs in range(2):
                    row = g * QW + qs * 128
                    for dh in range(2):
                        st = outp.tile([128, 512], F32, tag="numst",
                                       name=f"st_{g}_{qs}_{dh}")
                        if dh == 0:
                            nc.vector.tensor_copy(st[:], av[qs * 2 + dh][:])
                        else:
                            nc.scalar.copy(st[:], av[qs * 2 + dh][:])
                        eng = nc.sync if dh == 0 else nc.scalar
                        eng.dma_start(
                            num.ap()[row:row + 128, dh * 512:(dh + 1) * 512], st[:])
                dtmp = outp.tile([1, QW], F32, tag="numst", name=f"dtmp_{g}")
                nc.vector.tensor_copy(dtmp[:], dn[:])
                nc.gpsimd.dma_start(den.ap()[:, g * QW:(g + 1) * QW], dtmp[:])

    nc.compile()
    return nc


def _chunks(a, w):
    """[1024, n] (d-major) -> [n//w, 128, DB, w] chunk-major tile layout:
    element (c, p, db, j) = a[db*128 + p, c*w + j]."""
    d, n = a.shape
    return np.ascontiguousarray(
        a.reshape(DB, 128, n // w, w).transpose(2, 1, 0, 3))


def make_core_inputs(x, wqT, wkT, wvT, seq):
    """Per-core in_maps for batch elements of x [B, seq, d]."""
    NKB = seq // 256
    wq_d = _chunks(wqT, 128).astype(BF16_NP)
    wk_d = _chunks(wkT, 128).astype(BF16_NP)
    wv_d = _chunks(wvT, 128).astype(BF16_NP)
    masks = []
    for h in range(2):
        kk = np.arange(128)[:, None]
        qq = np.arange(QW)[None, :]
        masks.append((kk + 128 * h <= qq).astype(BF16_NP))
    in_maps = []
    for b in range(x.shape[0]):
        xT = np.ascontiguousarray(x[b].T)  # [d, seq]
        xq_d = _chunks(xT, QW).astype(BF16_NP)
        for h in range(2):
            cols = np.concatenate(
                [np.arange((2 * t + h) * 128, (2 * t + h + 1) * 128)
                 for t in range(NKB)])
            xk_d = _chunks(np.ascontiguousarray(xT[:, cols]),
                           min(256, seq // 2)).astype(BF16_NP)
            in_maps.append({
                "xq": xq_d, "xk": xk_d, "wq": wq_d, "wk": wk_d, "wv": wv_d,
                "mask": masks[h],
            })
    return in_maps


_prog_cache = {}


def _get_program(seq, num_devices):
    key = (seq, num_devices)
    if key not in _prog_cache:
        _prog_cache[key] = build_program(seq, num_devices)
    return _prog_cache[key]


def combine_partials(results, batch, seq):
    out = np.empty((batch, seq, D), dtype=np.float32)
    for b in range(batch):
        r0, r1 = results[2 * b], results[2 * b + 1]
        num = r0["num"].astype(np.float64) + r1["num"].astype(np.float64)
        den_flat = (r0["den"].astype(np.float64)
                    + r1["den"].astype(np.float64)).reshape(-1)
        out[b] = (num / den_flat[:, None]).astype(np.float32)
    return out


def kernel(x, Wq, Wk, Wv):
    x = np.asarray(x, dtype=np.float32)
    batch, seq, d = x.shape
    assert d == D
    wqT = np.ascontiguousarray(np.asarray(Wq, dtype=np.float32).T)
    wkT = np.ascontiguousarray(np.asarray(Wk, dtype=np.float32).T)
    wvT = np.ascontiguousarray(np.asarray(Wv, dtype=np.float32).T)
    n_cores = 2 * batch
    nc = _get_program(seq, n_cores)
    in_maps = make_core_inputs(x, wqT, wkT, wvT, seq)
    res = run_bass_kernel_spmd(nc, in_maps, core_ids=list(range(n_cores)))
    return combine_partials(res.results, batch, seq)
